# revision 6
# baseline (speedup 1.0000x reference)
# Bahdanau-attention kernel for TRN2, data-parallel over batch across 8 NeuronCores.
#
# reference math (B=16, S=2048, H=1024):
#   h_proj = hidden @ W[:, :H].T                      [B, H]
#   e_proj = einsum('bsh,gh->bsg', enc, W[:, H:])     [B, S, H]
#   scores = tanh(h_proj[:,None,:] + e_proj + b)      [B, S, H]
#   logits = scores @ v                               [B, S]
#   out    = softmax(logits, -1)[:, None, :]          [B, 1, S]
#
# Per-core layout (2 batches/core). The e_proj GEMM is the hard floor:
# 256 DoubleRow fp8 MMs x 512 cols ~= 55.3us at the 157 TF/s fp8 peak
# (measured: one 512-col DR MM issues every ~216ns). Everything else is
# arranged to hide under it:
#   - h_proj + b precomputed on host (hb), uploaded as a [128, GT, BPC] bias
#   - tanh on ScalarE per (chunk, j) with per-partition bias, fp8 scores out
#   - v-dot per chunk: 4 DoubleRow MMs over j-pairs into an M=1 psum row
#     (output cost is N-bound, so M=1 costs the same as M=128 but leaves a
#     single clean logits row)
#   - softmax is computed ON HOST from the DMA'd logits (v is uploaded
#     pre-scaled by VSCALE for fp8 range; host divides it back out).
#     This removes the fp32 mask-MMs from the PE stream and the
#     exp/reciprocal/scale tail chain after the last matmul.
#   - DVE copies each chunk's psum logits row into a per-batch [1, S] sbuf
#     tile; one 8KB DMA per batch ships it out on the (idle) gpsimd queue.
#   - enc DRAM layout is [bb, p, chunk, k, s]: each (p, chunk) is a
#     contiguous 4KB run so DMA packets are fat (the old s-sliced layout
#     produced 512B packets at ~15GB/s per engine).
#   - gating transfers (enc chunk 0, we j0/j1) are spread over the sync,
#     scalar and vector queues (all HW-dynamic); the slow-starting gpsimd
#     queue only carries late-deadline bulk (odd we tiles, all of enc bb1).
#   - warm-up: a short run of dummy MMs on junk data bridges the PE from
#     preamble end to first real data so the clock stays ramped; a tiny
#     tanh preloads the ACT table set before the first real tanh.

import numpy as np
import ml_dtypes

import concourse.bass as bass
import concourse.mybir as mybir
import concourse.tile as tile
from concourse import bacc
from concourse.bass_utils import run_bass_kernel_spmd

B, S, H = 16, 2048, 1024
NCORES = 8
BPC = B // NCORES          # batches per core
KT = H // 128              # contraction tiles
GT = H // 128              # output (g) tiles
SBLK = 512                 # s-chunk (one PSUM bank of f32)
NSB = S // SBLK

BF16 = mybir.dt.bfloat16
F32 = mybir.dt.float32
FP8 = mybir.dt.float8e4
WSCALE = 32.0              # W_e pre-scaled into fp8's sweet range; undone in tanh's scale
VSCALE = 16.0              # v pre-scaled; undone in the host softmax
DR = mybir.MatmulPerfMode.DoubleRow
DUMMY_MMS = 16

_CACHE = {}


def _build():
    nc = bacc.Bacc("TRN2", target_bir_lowering=False, debug=False, num_devices=NCORES)

    encT_d = nc.dram_tensor("encT", [BPC, 128, NSB, KT, SBLK], FP8, kind="ExternalInput")
    we_d = nc.dram_tensor("we", [128, GT, KT, 128], FP8, kind="ExternalInput")
    hb_d = nc.dram_tensor("hb", [128, GT, BPC], F32, kind="ExternalInput")
    vvec_d = nc.dram_tensor("vvec", [128, GT, 16], FP8, kind="ExternalInput")
    out_d = nc.dram_tensor("out", [BPC, NSB, SBLK], F32, kind="ExternalOutput")

    ACT = mybir.ActivationFunctionType

    with tile.TileContext(nc) as tc:
        with (
            tc.tile_pool(name="const", bufs=1) as constp,
            tc.tile_pool(name="wp", bufs=1) as wp,
            tc.tile_pool(name="encp", bufs=1) as encp,
            tc.tile_pool(name="scp", bufs=2) as scp,
            tc.tile_pool(name="smallp", bufs=2) as smallp,
            tc.tile_pool(name="mps", bufs=4, space="PSUM") as mps,
            tc.tile_pool(name="lps", bufs=2, space="PSUM") as lps,
        ):
            # --- warm-up: junk memset on DVE feeds (a) an ACT-table-preloading
            # tanh and (b) dummy MMs that keep the PE clock ramped until real
            # data lands.
            junk = constp.tile([128, 2, 128], FP8, tag="junk")
            nc.vector.memset(junk[:], 0)
            jact = constp.tile([1, 1], F32, tag="jact")
            nc.scalar.activation(jact[:], junk[0:1, 0, 0:1], ACT.Tanh)
            dummy = mps.tile([128, SBLK], F32, tag="mp", name="dummy")
            for _ in range(DUMMY_MMS):
                nc.tensor.matmul(
                    dummy[:, 0:128], junk[:], junk[:],
                    start=True, stop=True, perf_mode=DR,
                )

            # --- tiles ---
            we_sb = [None] * GT

            def load_we(j, eng):
                t = wp.tile([128, KT, 128], FP8, name=f"we{j}", tag=f"we{j}")
                eng.dma_start(out=t[:], in_=we_d[:, j])
                we_sb[j] = t

            enc_sb = [
                encp.tile([128, NSB, KT, SBLK], FP8, name=f"enc{bb}", tag=f"enc{bb}")
                for bb in range(BPC)
            ]

            def load_enc(bb, c, klo, khi, eng):
                eng.dma_start(
                    out=enc_sb[bb][:, c, klo:khi, :],
                    in_=encT_d[bb][:, c, klo:khi, :],
                )

            hb_sb = constp.tile([128, GT, BPC], F32, tag="hb")
            v_sb = constp.tile([128, GT, 16], FP8, tag="vvec")

            # --- DMA kicks. ONE hardware-dynamic queue (sync): aggregate DMA
            # bandwidth is ~245GB/s regardless of queue count (16 engines x
            # ~15GB/s), so a single queue with descriptors in exact
            # consumption order beats any multi-queue split -- and extra
            # dynamic queues measurably lengthen the framework preamble.
            # enc chunk-0 quarter-kicks are interleaved with the first weight
            # tiles so the j0 MM group starts streaming as soon as k01+we0
            # land; subtile deps let each kp MM fire on its own quarter.
            load_enc(0, 0, 0, 2, nc.sync)
            load_we(0, nc.sync)
            load_enc(0, 0, 2, 4, nc.sync)
            load_we(1, nc.sync)
            load_enc(0, 0, 4, 6, nc.sync)
            load_we(2, nc.sync)
            load_enc(0, 0, 6, 8, nc.sync)
            nc.sync.dma_start(out=hb_sb[:], in_=hb_d[:])
            load_we(3, nc.sync)
            load_we(4, nc.sync)
            load_we(5, nc.sync)
            load_we(6, nc.sync)
            load_we(7, nc.sync)
            nc.sync.dma_start(out=v_sb[:], in_=vvec_d[:])
            nc.sync.dma_start(out=enc_sb[0][:, 1], in_=encT_d[0][:, 1])
            nc.sync.dma_start(out=enc_sb[0][:, 2], in_=encT_d[0][:, 2])
            nc.sync.dma_start(out=enc_sb[0][:, 3], in_=encT_d[0][:, 3])
            nc.sync.dma_start(out=enc_sb[1][:], in_=encT_d[1][:])

            # --- main loop ---
            sc_all = [
                scp.tile([128, GT, S], FP8, name=f"scores{bb}",
                         tag=f"scores{bb}", bufs=1)
                for bb in range(BPC)
            ]
            logit_sb = [
                smallp.tile([1, S], F32, name=f"logit{bb}", tag=f"logit{bb}",
                            bufs=1)
                for bb in range(BPC)
            ]

            for bb in range(BPC):
                for c in range(NSB):
                    sl = slice(c * SBLK, (c + 1) * SBLK)
                    for j in range(GT):
                        mp = mps.tile([128, SBLK], F32, tag="mp", name=f"mp{j}")
                        for kp in range(KT // 2):
                            nc.tensor.matmul(
                                mp[:],
                                we_sb[j][:, 2 * kp : 2 * kp + 2, :],
                                enc_sb[bb][:, c, 2 * kp : 2 * kp + 2, :],
                                start=(kp == 0),
                                stop=(kp == KT // 2 - 1),
                                perf_mode=DR,
                            )
                        nc.scalar.activation(
                            sc_all[bb][:, j, sl], mp[:], ACT.Tanh,
                            bias=hb_sb[:, j, bb : bb + 1],
                            scale=1.0 / WSCALE,
                        )
                    # v-dot: 4 DoubleRow MMs over j-pairs into a single psum
                    # row (M=1); the logits land on partition 0.
                    lp = lps.tile([1, SBLK], F32, tag="lp")
                    for jp in range(GT // 2):
                        nc.tensor.matmul(
                            lp[:],
                            v_sb[:, 2 * jp : 2 * jp + 2, 0:1],
                            sc_all[bb][:, 2 * jp : 2 * jp + 2, sl],
                            start=(jp == 0),
                            stop=(jp == GT // 2 - 1),
                            perf_mode=DR,
                        )
                    nc.vector.tensor_copy(logit_sb[bb][:, sl], lp[:])
                # one 8KB DMA per batch; the sync queue is empty by the
                # time these fire.
                nc.sync.dma_start(out=out_d[bb], in_=logit_sb[bb][:])

    nc.compile()
    return nc


def _get_nc():
    if "nc" not in _CACHE:
        _CACHE["nc"] = _build()
    return _CACHE["nc"]


def _make_in_maps(hidden, encoder_outputs, W, b, v):
    fp8 = ml_dtypes.float8_e4m3
    WT = np.ascontiguousarray(W.T)  # [2H, H]; WT[hin, gout]
    w_tiles = WT.reshape(2, KT, 128, GT, 128).transpose(0, 2, 3, 1, 4)  # [half, p, j, k, m]
    we_host = np.ascontiguousarray(w_tiles[1] * WSCALE).astype(fp8)

    # h_proj + b on host in f64: a [B, H] bias, 0.03% of the FLOPs.
    hb_full = (
        hidden.astype(np.float64) @ W[:, :H].astype(np.float64).T
        + b.astype(np.float64)
    ).astype(np.float32)  # [B, H]

    v_host = np.zeros((128, GT, 16), dtype=fp8)
    v_host[:, :, 0] = (v.reshape(GT, 128).T * VSCALE).astype(fp8)

    in_maps = []
    for i in range(NCORES):
        hs = hb_full[BPC * i : BPC * (i + 1)]  # [BPC, H]
        hb_host = np.ascontiguousarray(
            hs.reshape(BPC, GT, 128).transpose(2, 1, 0)
        ).astype(np.float32)  # [128, GT, BPC]
        es = encoder_outputs[BPC * i : BPC * (i + 1)]  # [BPC, S, H]
        # [bb, p, c, k, s]: each (p, c) is a contiguous KT*SBLK = 4KB run.
        eT = np.ascontiguousarray(
            es.reshape(BPC, NSB, SBLK, KT, 128).transpose(0, 4, 1, 3, 2)
        ).astype(fp8)
        in_maps.append(
            {
                "encT": eT,
                "we": we_host,
                "hb": hb_host,
                "vvec": v_host,
            }
        )
    return in_maps


def _run(in_maps, **kwargs):
    nc = _get_nc()
    try:
        return run_bass_kernel_spmd(
            nc, in_maps, core_ids=list(range(NCORES)), **kwargs
        )
    except Exception:
        # A first execution right after NEFF load has been seen to wedge the
        # device once; it recovers after a short pause. Retry once.
        import time as _time

        _time.sleep(20)
        return run_bass_kernel_spmd(
            nc, in_maps, core_ids=list(range(NCORES)), **kwargs
        )


def kernel(hidden, encoder_outputs, W, b, v):
    hidden = np.asarray(hidden, dtype=np.float32)
    encoder_outputs = np.asarray(encoder_outputs, dtype=np.float32)
    W = np.asarray(W, dtype=np.float32)
    b = np.asarray(b, dtype=np.float32)
    v = np.asarray(v, dtype=np.float32)

    in_maps = _make_in_maps(hidden, encoder_outputs, W, b, v)
    res = _run(in_maps)
    logits = np.concatenate(
        [
            np.asarray(res.results[i]["out"], dtype=np.float32).reshape(BPC, S)
            for i in range(NCORES)
        ],
        axis=0,
    ).astype(np.float64) / VSCALE  # [B, S]
    # softmax on host (float64; ~0.003% of the kernel FLOPs)
    logits -= logits.max(axis=-1, keepdims=True)
    e = np.exp(logits)
    out = (e / e.sum(axis=-1, keepdims=True)).astype(np.float32)
    return out.reshape(B, 1, S)


# revision 8
# speedup vs baseline: 1.0410x; 1.0410x over previous
# Bahdanau-attention kernel for TRN2, data-parallel over batch across 8 NeuronCores.
#
# reference math (B=16, S=2048, H=1024):
#   h_proj = hidden @ W[:, :H].T                      [B, H]
#   e_proj = einsum('bsh,gh->bsg', enc, W[:, H:])     [B, S, H]
#   scores = tanh(h_proj[:,None,:] + e_proj + b)      [B, S, H]
#   logits = scores @ v                               [B, S]
#   out    = softmax(logits, -1)[:, None, :]          [B, 1, S]
#
# Per-core layout (2 batches/core). The e_proj GEMM is the hard floor:
# 256 DoubleRow fp8 MMs x 512 cols ~= 55.3us at the 157 TF/s fp8 peak
# (measured: one 512-col DR MM issues every ~216ns). Everything else is
# arranged to hide under it:
#   - h_proj + b precomputed on host (hb), uploaded as a [128, GT, BPC] bias
#   - tanh on ScalarE per (chunk, j) with per-partition bias, fp8 scores out
#   - v-dot per chunk: 4 DoubleRow MMs over j-pairs into an M=1 psum row
#     (output cost is N-bound, so M=1 costs the same as M=128 but leaves a
#     single clean logits row)
#   - softmax is computed ON HOST from the DMA'd logits (v is uploaded
#     pre-scaled by VSCALE for fp8 range; host divides it back out).
#     This removes the fp32 mask-MMs from the PE stream and the
#     exp/reciprocal/scale tail chain after the last matmul.
#   - DVE copies each chunk's psum logits row into a per-batch [1, S] sbuf
#     tile; one 8KB DMA per batch ships it out on the (idle) gpsimd queue.
#   - enc DRAM layout is [bb, p, chunk, k, s]: each (p, chunk) is a
#     contiguous 4KB run so DMA packets are fat (the old s-sliced layout
#     produced 512B packets at ~15GB/s per engine).
#   - gating transfers (enc chunk 0, we j0/j1) are spread over the sync,
#     scalar and vector queues (all HW-dynamic); the slow-starting gpsimd
#     queue only carries late-deadline bulk (odd we tiles, all of enc bb1).
#   - warm-up: a short run of dummy MMs on junk data bridges the PE from
#     preamble end to first real data so the clock stays ramped; a tiny
#     tanh preloads the ACT table set before the first real tanh.

import numpy as np
import ml_dtypes

import concourse.bass as bass
import concourse.mybir as mybir
import concourse.tile as tile
from concourse import bacc
from concourse.bass_utils import run_bass_kernel_spmd

B, S, H = 16, 2048, 1024
NCORES = 8
BPC = B // NCORES          # batches per core
KT = H // 128              # contraction tiles
GT = H // 128              # output (g) tiles
SBLK = 512                 # s-chunk (one PSUM bank of f32)
NSB = S // SBLK

BF16 = mybir.dt.bfloat16
F32 = mybir.dt.float32
FP8 = mybir.dt.float8e4
WSCALE = 32.0              # W_e pre-scaled into fp8's sweet range; undone in tanh's scale
VSCALE = 16.0              # v pre-scaled; undone in the host softmax
DR = mybir.MatmulPerfMode.DoubleRow
DUMMY_MMS = 36

_CACHE = {}


def _build():
    nc = bacc.Bacc("TRN2", target_bir_lowering=False, debug=False, num_devices=NCORES)

    encT_d = nc.dram_tensor("encT", [BPC, 128, NSB, KT, SBLK], FP8, kind="ExternalInput")
    we_d = nc.dram_tensor("we", [128, GT, KT, 128], FP8, kind="ExternalInput")
    hb_d = nc.dram_tensor("hb", [128, GT, BPC], F32, kind="ExternalInput")
    vvec_d = nc.dram_tensor("vvec", [128, GT, 16], FP8, kind="ExternalInput")
    out_d = nc.dram_tensor("out", [BPC, NSB, SBLK], F32, kind="ExternalOutput")

    ACT = mybir.ActivationFunctionType

    with tile.TileContext(nc) as tc:
        with (
            tc.tile_pool(name="const", bufs=1) as constp,
            tc.tile_pool(name="wp", bufs=1) as wp,
            tc.tile_pool(name="encp", bufs=1) as encp,
            tc.tile_pool(name="scp", bufs=2) as scp,
            tc.tile_pool(name="smallp", bufs=2) as smallp,
            tc.tile_pool(name="mps", bufs=4, space="PSUM") as mps,
            tc.tile_pool(name="lps", bufs=2, space="PSUM") as lps,
        ):
            # --- warm-up: junk memset on DVE feeds (a) an ACT-table-preloading
            # tanh and (b) dummy MMs that keep the PE clock ramped until real
            # data lands.
            junk = constp.tile([128, 2, 128], FP8, tag="junk")
            nc.vector.memset(junk[:], 0)
            jact = constp.tile([1, 1], F32, tag="jact")
            nc.scalar.activation(jact[:], junk[0:1, 0, 0:1], ACT.Tanh)
            dummy = mps.tile([128, SBLK], F32, tag="mp", name="dummy")
            for _ in range(DUMMY_MMS):
                nc.tensor.matmul(
                    dummy[:, 0:128], junk[:], junk[:],
                    start=True, stop=True, perf_mode=DR,
                )

            # --- tiles ---
            we_sb = [None] * GT

            def load_we(j, eng):
                t = wp.tile([128, KT, 128], FP8, name=f"we{j}", tag=f"we{j}")
                eng.dma_start(out=t[:], in_=we_d[:, j])
                we_sb[j] = t

            enc_sb = [
                encp.tile([128, NSB, KT, SBLK], FP8, name=f"enc{bb}", tag=f"enc{bb}")
                for bb in range(BPC)
            ]

            def load_enc(bb, c, klo, khi, eng):
                eng.dma_start(
                    out=enc_sb[bb][:, c, klo:khi, :],
                    in_=encT_d[bb][:, c, klo:khi, :],
                )

            hb_sb = constp.tile([128, GT, BPC], F32, tag="hb")
            v_sb = constp.tile([128, GT, 16], FP8, tag="vvec")

            # --- DMA kicks. Two queues (sync HW-dynamic + gpsimd) run
            # concurrently at ~245GB/s aggregate; each queue's descriptor
            # order follows consumption order. The gating set (enc c0 +
            # we0) is split across both queues; weights alternate between
            # queues so each we_j lands well before its j-group; full-chunk
            # enc kicks give contiguous 4KB-per-partition runs (fat packets
            # sustain ~300GB/s vs ~130GB/s for 1KB packets).
            # sync queue:
            load_we(0, nc.sync)
            load_enc(0, 0, 0, 4, nc.sync)
            nc.sync.dma_start(out=hb_sb[:], in_=hb_d[:])
            load_we(2, nc.sync)
            load_we(4, nc.sync)
            load_we(6, nc.sync)
            nc.sync.dma_start(out=enc_sb[0][:, 1], in_=encT_d[0][:, 1])
            nc.sync.dma_start(out=enc_sb[0][:, 3], in_=encT_d[0][:, 3])
            nc.sync.dma_start(out=enc_sb[1][:, 1], in_=encT_d[1][:, 1])
            nc.sync.dma_start(out=enc_sb[1][:, 3], in_=encT_d[1][:, 3])
            # gpsimd queue (starts ~1.5us later; carries the upper k-half of
            # the gating chunk plus odd weights and the other chunk bulk):
            load_enc(0, 0, 4, 8, nc.gpsimd)
            load_we(1, nc.gpsimd)
            load_we(3, nc.gpsimd)
            load_we(5, nc.gpsimd)
            load_we(7, nc.gpsimd)
            nc.gpsimd.dma_start(out=v_sb[:], in_=vvec_d[:])
            nc.gpsimd.dma_start(out=enc_sb[0][:, 2], in_=encT_d[0][:, 2])
            nc.gpsimd.dma_start(out=enc_sb[1][:, 0], in_=encT_d[1][:, 0])
            nc.gpsimd.dma_start(out=enc_sb[1][:, 2], in_=encT_d[1][:, 2])

            # --- main loop ---
            sc_all = [
                scp.tile([128, GT, S], FP8, name=f"scores{bb}",
                         tag=f"scores{bb}", bufs=1)
                for bb in range(BPC)
            ]
            logit_sb = [
                smallp.tile([1, S], F32, name=f"logit{bb}", tag=f"logit{bb}",
                            bufs=1)
                for bb in range(BPC)
            ]

            for bb in range(BPC):
                for c in range(NSB):
                    sl = slice(c * SBLK, (c + 1) * SBLK)
                    for j in range(GT):
                        mp = mps.tile([128, SBLK], F32, tag="mp", name=f"mp{j}")
                        for kp in range(KT // 2):
                            nc.tensor.matmul(
                                mp[:],
                                we_sb[j][:, 2 * kp : 2 * kp + 2, :],
                                enc_sb[bb][:, c, 2 * kp : 2 * kp + 2, :],
                                start=(kp == 0),
                                stop=(kp == KT // 2 - 1),
                                perf_mode=DR,
                            )
                        nc.scalar.activation(
                            sc_all[bb][:, j, sl], mp[:], ACT.Tanh,
                            bias=hb_sb[:, j, bb : bb + 1],
                            scale=1.0 / WSCALE,
                        )
                    # v-dot: 4 DoubleRow MMs over j-pairs into a single psum
                    # row (M=1); the logits land on partition 0.
                    lp = lps.tile([1, SBLK], F32, tag="lp")
                    for jp in range(GT // 2):
                        nc.tensor.matmul(
                            lp[:],
                            v_sb[:, 2 * jp : 2 * jp + 2, 0:1],
                            sc_all[bb][:, 2 * jp : 2 * jp + 2, sl],
                            start=(jp == 0),
                            stop=(jp == GT // 2 - 1),
                            perf_mode=DR,
                        )
                    nc.vector.tensor_copy(logit_sb[bb][:, sl], lp[:])
                # one 8KB DMA per batch; the sync queue is empty by the
                # time these fire.
                nc.sync.dma_start(out=out_d[bb], in_=logit_sb[bb][:])

    nc.compile()
    return nc


def _get_nc():
    if "nc" not in _CACHE:
        _CACHE["nc"] = _build()
    return _CACHE["nc"]


def _make_in_maps(hidden, encoder_outputs, W, b, v):
    fp8 = ml_dtypes.float8_e4m3
    WT = np.ascontiguousarray(W.T)  # [2H, H]; WT[hin, gout]
    w_tiles = WT.reshape(2, KT, 128, GT, 128).transpose(0, 2, 3, 1, 4)  # [half, p, j, k, m]
    we_host = np.ascontiguousarray(w_tiles[1] * WSCALE).astype(fp8)

    # h_proj + b on host in f64: a [B, H] bias, 0.03% of the FLOPs.
    hb_full = (
        hidden.astype(np.float64) @ W[:, :H].astype(np.float64).T
        + b.astype(np.float64)
    ).astype(np.float32)  # [B, H]

    v_host = np.zeros((128, GT, 16), dtype=fp8)
    v_host[:, :, 0] = (v.reshape(GT, 128).T * VSCALE).astype(fp8)

    in_maps = []
    for i in range(NCORES):
        hs = hb_full[BPC * i : BPC * (i + 1)]  # [BPC, H]
        hb_host = np.ascontiguousarray(
            hs.reshape(BPC, GT, 128).transpose(2, 1, 0)
        ).astype(np.float32)  # [128, GT, BPC]
        es = encoder_outputs[BPC * i : BPC * (i + 1)]  # [BPC, S, H]
        # [bb, p, c, k, s]: each (p, c) is a contiguous KT*SBLK = 4KB run.
        eT = np.ascontiguousarray(
            es.reshape(BPC, NSB, SBLK, KT, 128).transpose(0, 4, 1, 3, 2)
        ).astype(fp8)
        in_maps.append(
            {
                "encT": eT,
                "we": we_host,
                "hb": hb_host,
                "vvec": v_host,
            }
        )
    return in_maps


def _run(in_maps, **kwargs):
    nc = _get_nc()
    try:
        return run_bass_kernel_spmd(
            nc, in_maps, core_ids=list(range(NCORES)), **kwargs
        )
    except Exception:
        # A first execution right after NEFF load has been seen to wedge the
        # device once; it recovers after a short pause. Retry once.
        import time as _time

        _time.sleep(20)
        return run_bass_kernel_spmd(
            nc, in_maps, core_ids=list(range(NCORES)), **kwargs
        )


def kernel(hidden, encoder_outputs, W, b, v):
    hidden = np.asarray(hidden, dtype=np.float32)
    encoder_outputs = np.asarray(encoder_outputs, dtype=np.float32)
    W = np.asarray(W, dtype=np.float32)
    b = np.asarray(b, dtype=np.float32)
    v = np.asarray(v, dtype=np.float32)

    in_maps = _make_in_maps(hidden, encoder_outputs, W, b, v)
    res = _run(in_maps)
    logits = np.concatenate(
        [
            np.asarray(res.results[i]["out"], dtype=np.float32).reshape(BPC, S)
            for i in range(NCORES)
        ],
        axis=0,
    ).astype(np.float64) / VSCALE  # [B, S]
    # softmax on host (float64; ~0.003% of the kernel FLOPs)
    logits -= logits.max(axis=-1, keepdims=True)
    e = np.exp(logits)
    out = (e / e.sum(axis=-1, keepdims=True)).astype(np.float32)
    return out.reshape(B, 1, S)
